# revision 15
# baseline (speedup 1.0000x reference)
"""Trainium2 Bass kernel: 3-layer edge-weighted GraphSAGE (Cluster-GCN style).

Strategy (8 NeuronCores, SPMD):
  - Nodes padded to NPAD = 8*SHARD, shard k = rows [k*SHARD, (k+1)*SHARD).
  - Edges bucketed by (dst tile-PAIR, src half); per pair the incoming edges
    are gathered (src rows) with dma_gather, then aggregated with a one-hot
    matmul into PSUM: S[e, dst_local_in_pair] = w'_e built ON HOST (bf16)
    and DMA-streamed per pair; w' = edge_attr / max(indeg, 1) folds the
    mean.  Pair-width (256) S matrices mean a chunk can mix edges of both
    tiles, minimizing ceil-padding.
  - Layer outputs are AllGather'ed so every core holds the full feature
    table for the next layer's gather.  Layer 3 projects h2 @ Wl2 first
    (8 cols) so its gather moves 256B/edge instead of 1KB/edge.
  - dma_gather uses int16 indices, so tables are split A/B at local row
    SLA=4096 (table-A = 8*4096 = 32768 rows, exactly int16 range).  Each
    AllGather is issued as two collectives (A after pair 15, B at layer
    end) so transfers overlap the gather stream; L2/L3 prefetch A-bucket
    gathers of later pairs while the B-table collective is in flight.
  - Tiles are dealt to (core, slot) positions by edge-count rank so
    same-slot buckets have near-equal counts across cores (less NAB pad).
  - bf16 matmul operands (fp32 PSUM accumulation) for full PE rate and
    half gather bandwidth.
"""
import numpy as np

import concourse.bacc as bacc
import concourse.tile as tile
from concourse import mybir
from concourse.bass_utils import run_bass_kernel_spmd
from concourse.masks import make_identity

try:
    from ml_dtypes import bfloat16 as np_bf16
except ImportError:  # pragma: no cover
    import jax.numpy as _jnp
    np_bf16 = _jnp.bfloat16

F32 = mybir.dt.float32
BF16 = mybir.dt.bfloat16
I16 = mybir.dt.int16
P = 128
Alu = mybir.AluOpType
Act = mybir.ActivationFunctionType


class Cfg:
    def __init__(self, n_nodes=50000, n_edges=800000, dims=(128, 256, 256, 8),
                 ncores=8, bf16=True):
        self.N, self.E, self.D, self.NC = n_nodes, n_edges, dims, ncores
        self.SHARD = ((n_nodes + ncores * P - 1) // (ncores * P)) * P
        self.NPAD = self.SHARD * ncores
        self.TPC = self.SHARD // P
        self.NPAIR = (self.TPC + 1) // 2
        self.HALF = self.NPAD // 2
        self.SLA = 4096                      # per-core A rows (pairs 0-15)
        self.SLB = self.SHARD - self.SLA     # per-core B rows
        self.NTA = self.NC * self.SLA        # table-A rows (<= 32768)
        self.NTB = self.NC * self.SLB        # table-B rows
        assert self.NTA <= 32768 and self.NTB < 32768
        assert dims[0] == P and dims[1] % P == 0 and dims[2] % P == 0
        self.bf16 = bf16
        # L3 gather table row width (256B rows)
        self.EL3 = 128 if bf16 else 64

    def key(self):
        return (self.N, self.E, self.D, self.NC, self.bf16)


class Plan:
    def __init__(self, cfg, NAB):
        self.NAB = NAB  # [NPAIR, 2]
        self.pairs = [list(range(2 * p, min(cfg.TPC, 2 * p + 2)))
                      for p in range(cfg.NPAIR)]
        self.cb = np.zeros((cfg.NPAIR, 2), np.int64)
        self.gbase = []
        c = 0
        for p in range(cfg.NPAIR):
            self.gbase.append(c)
            self.cb[p, 0] = c
            c += NAB[p, 0]
            self.cb[p, 1] = c
            c += NAB[p, 1]
        self.CT = c


def balance_tiles(cfg, dst):
    """Assign node tiles to (core, slot) so that same-slot tiles have
    similar edge counts across cores (shrinks max-over-core NAB pad)."""
    ntile = cfg.NC * cfg.TPC
    tcnt = np.bincount(dst >> 7, minlength=ntile)
    rank = np.argsort(-tcnt, kind="stable")
    assign = np.empty((cfg.NC, cfg.TPC), np.int64)
    for s in range(cfg.TPC):
        octet = rank[s * cfg.NC:(s + 1) * cfg.NC]
        if s % 2:
            octet = octet[::-1]
        assign[:, s] = octet
    tile_core = np.empty(ntile, np.int64)
    tile_slot = np.empty(ntile, np.int64)
    for k in range(cfg.NC):
        tile_core[assign[k]] = k
        tile_slot[assign[k]] = np.arange(cfg.TPC)
    return assign, tile_core, tile_slot


def host_prep(cfg, x, edge_index, edge_attr):
    src = edge_index[0].astype(np.int64)
    dst = edge_index[1].astype(np.int64)
    cnt = np.bincount(dst, minlength=cfg.N).astype(np.float32)
    wp = (edge_attr.astype(np.float32)
          / np.maximum(cnt, 1.0)[dst]).astype(np.float32)

    assign, tile_core, tile_slot = balance_tiles(cfg, dst)
    # node u -> (core, local slot-major row)
    nloc = tile_slot[dst >> 7] * P + (dst & (P - 1))
    ncore = tile_core[dst >> 7]
    sloc = tile_slot[src >> 7] * P + (src & (P - 1))
    score = tile_core[src >> 7]
    segkey = (ncore * cfg.NPAIR + nloc // (2 * P)) * 2 + (sloc >= cfg.SLA)
    # secondary sort by src for DMA locality within each bucket
    order = np.lexsort((src, segkey))
    ssrc, sdst, swp = sloc[order], nloc[order], wp[order]
    sscore = score[order]
    nseg = cfg.NC * cfg.NPAIR * 2
    seg_counts = np.bincount(segkey, minlength=nseg)
    seg_start = np.zeros(nseg + 1, np.int64)
    seg_start[1:] = np.cumsum(seg_counts)
    sc = seg_counts.reshape(cfg.NC, cfg.NPAIR, 2)
    NAB = np.maximum(
        np.ceil(sc / P).astype(np.int64).max(axis=0), 1)  # [NPAIR, 2]
    plan = Plan(cfg, NAB)
    CT = plan.CT

    idx_arr = np.zeros((cfg.NC, 16, CT * 8), np.int16)
    s_arr = np.zeros((cfg.NC, CT, P, 2 * P), np.float32)
    for k in range(cfg.NC):
        for p in range(cfg.NPAIR):
            for h in (0, 1):
                si = (k * cfg.NPAIR + p) * 2 + h
                i0, n = seg_start[si], seg_counts[si]
                if n == 0:
                    continue
                sk = sscore[i0:i0 + n]
                sl = ssrc[i0:i0 + n]
                if h == 0:
                    rows = (sk * cfg.SLA + sl).astype(np.int16)
                else:
                    rows = (sk * cfg.SLB + (sl - cfg.SLA)).astype(np.int16)
                dl = (sdst[i0:i0 + n] - p * 2 * P).astype(np.int64)
                cb = plan.cb[p, h]
                j = np.arange(n)
                s_arr[k, cb + j // P, j % P, dl] = swp[i0:i0 + n]
                idx_arr[k, j % 16, cb * 8 + j // 16] = rows
    idx_full = np.tile(idx_arr, (1, 8, 1))
    # S layout for DMA: [e(partition), CT, dst] so a pair's chunks are a
    # contiguous free-dim slice per partition.
    s_full = np.ascontiguousarray(
        s_arr.transpose(0, 2, 1, 3)).astype(np_bf16)
    return plan, idx_full, s_full, assign


def build_nc(cfg, plan):
    d0, d1, d2, d3 = cfg.D
    H1, H2 = d1 // P, d2 // P
    DT = BF16 if cfg.bf16 else F32
    CT = plan.CT
    NAB = plan.NAB

    nc = bacc.Bacc("TRN2", target_bir_lowering=False, debug=False,
                   num_devices=cfg.NC, enable_asserts=False)

    xfullA = nc.dram_tensor("xfullA", [cfg.NTA, d0], DT, kind="ExternalInput")
    xfullB = nc.dram_tensor("xfullB", [cfg.NTB, d0], DT, kind="ExternalInput")
    xT_in = nc.dram_tensor("xT", [P, cfg.SHARD], DT, kind="ExternalInput")
    idx_in = nc.dram_tensor("idx", [P, CT * 8], I16, kind="ExternalInput")
    s_in = nc.dram_tensor("smat", [P, CT, 2 * P], DT, kind="ExternalInput")
    wl0_in = nc.dram_tensor("wl0", [P, d1], DT, kind="ExternalInput")
    wr0_in = nc.dram_tensor("wr0", [P, d1], DT, kind="ExternalInput")
    wl1_in = nc.dram_tensor("wl1", [P, H1, d2], DT, kind="ExternalInput")
    wr1_in = nc.dram_tensor("wr1", [P, H1, d2], DT, kind="ExternalInput")
    wl2_in = nc.dram_tensor("wl2", [P, H2, d3], DT, kind="ExternalInput")
    wr2_in = nc.dram_tensor("wr2", [P, H2, d3], DT, kind="ExternalInput")
    b0_in = nc.dram_tensor("b0", [1, d1], DT, kind="ExternalInput")
    b1_in = nc.dram_tensor("b1", [1, d2], DT, kind="ExternalInput")
    b2_in = nc.dram_tensor("b2", [1, d3], DT, kind="ExternalInput")
    out_t = nc.dram_tensor("out", [cfg.SHARD, d3], F32, kind="ExternalOutput")

    with tile.TileContext(nc) as tc:
        with (
            tc.tile_pool(name="const", bufs=1) as cp,
            tc.tile_pool(name="mt", bufs=5) as mp,
            tc.tile_pool(name="st", bufs=2) as sp,
            tc.tile_pool(name="wk", bufs=2) as wk,
            tc.tile_pool(name="psA", bufs=2, space="PSUM") as psA,
            tc.tile_pool(name="psB", bufs=2, space="PSUM") as psB,
            tc.tile_pool(name="dram", bufs=1, space="DRAM") as dr,
        ):
            h1own = dr.tile([cfg.SHARD, d1], DT)
            h1fullA = dr.tile([cfg.NTA, d1], DT, addr_space="Shared")
            h1fullB = dr.tile([cfg.NTB, d1], DT, addr_space="Shared")
            h1T = dr.tile([d1, cfg.SHARD], DT)
            h2T = dr.tile([d2, cfg.SHARD], DT)
            h2p = dr.tile([cfg.SHARD, cfg.EL3], DT)
            h2pfullA = dr.tile([cfg.NTA, cfg.EL3], DT, addr_space="Shared")
            h2pfullB = dr.tile([cfg.NTB, cfg.EL3], DT, addr_space="Shared")

            # ---- constants / parameters
            ident_f = cp.tile([P, P], F32)
            make_identity(nc, ident_f[:])
            if cfg.bf16:
                ident_b = cp.tile([P, P], BF16)
                nc.vector.tensor_copy(ident_b[:], ident_f[:])
                ident_dt = ident_b
            else:
                ident_dt = ident_f
            ones_t = cp.tile([1, P], DT)
            nc.vector.memset(ones_t[:], 1.0)
            idx_t = cp.tile([P, CT * 8], I16)
            nc.sync.dma_start(out=idx_t[:], in_=idx_in[:, :])
            xT_t = cp.tile([P, cfg.SHARD], DT)
            nc.sync.dma_start(out=xT_t[:], in_=xT_in[:, :])
            wl0_t = cp.tile([P, d1], DT)
            nc.sync.dma_start(out=wl0_t[:], in_=wl0_in[:, :])
            wr0_t = cp.tile([P, d1], DT)
            nc.sync.dma_start(out=wr0_t[:], in_=wr0_in[:, :])
            wl1_t = cp.tile([P, H1, d2], DT)
            nc.sync.dma_start(out=wl1_t[:], in_=wl1_in[:, :, :])
            wr1_t = cp.tile([P, H1, d2], DT)
            nc.sync.dma_start(out=wr1_t[:], in_=wr1_in[:, :, :])
            wl2_t = cp.tile([P, H2, d3], DT)
            nc.sync.dma_start(out=wl2_t[:], in_=wl2_in[:, :, :])
            wr2_t = cp.tile([P, H2, d3], DT)
            nc.sync.dma_start(out=wr2_t[:], in_=wr2_in[:, :, :])
            b0_t = cp.tile([1, d1], DT)
            nc.sync.dma_start(out=b0_t[:], in_=b0_in[:, :])
            b1_t = cp.tile([1, d2], DT)
            nc.sync.dma_start(out=b1_t[:], in_=b1_in[:, :])
            b2_t = cp.tile([1, d3], DT)
            nc.sync.dma_start(out=b2_t[:], in_=b2_in[:, :])

            MAXC = 4096 // P  # max chunks per dma_gather call

            def gather1(gi, h, table, elem, tag):
                ncH = int(NAB[gi, h])
                cb0 = int(plan.cb[gi, h])
                m_t = mp.tile([P, ncH, elem], DT, tag=tag,
                              name=f"m_{tag}_{gi}_{h}_{elem}")
                for s in range(0, ncH, MAXC):
                    n = min(MAXC, ncH - s)
                    nc.gpsimd.dma_gather(
                        m_t[:, s:s + n, :], table[:, :],
                        idx_t[:, (cb0 + s) * 8:(cb0 + s + n) * 8],
                        n * P, n * P, elem, single_packet=False)
                return m_t

            def chunk_of(gi, mA, mB, ci):
                nA = int(NAB[gi, 0])
                return mA[:, ci, :] if ci < nA else mB[:, ci - nA, :]

            def smats(gi):
                gb = plan.gbase[gi]
                ng = int(NAB[gi, 0] + NAB[gi, 1])
                s_t = sp.tile([P, ng, 2 * P], DT, tag="st")
                nc.sync.dma_start(out=s_t[:], in_=s_in[:, gb:gb + ng, :])
                return s_t

            # ---------------- Layer 1 ----------------
            for gi, tl in enumerate(plan.pairs):
                mA = gather1(gi, 0, xfullA, d0, "mtA")
                mB = gather1(gi, 1, xfullB, d0, "mtB")
                s_t = smats(gi)
                ng = int(NAB[gi, 0] + NAB[gi, 1])
                width = P * len(tl)
                aggT = psA.tile([P, 2 * P], F32, tag="agg0")
                for ci in range(ng):
                    nc.tensor.matmul(
                        out=aggT[:, 0:width], lhsT=chunk_of(gi, mA, mB, ci),
                        rhs=s_t[:, ci, 0:width],
                        start=(ci == 0), stop=(ci == ng - 1))
                meanT = wk.tile([P, 2 * P], DT, tag="meanT")
                nc.scalar.activation(meanT[:, 0:width], aggT[:, 0:width],
                                     Act.Copy)
                for j, t in enumerate(tl):
                    op_ = psB.tile([P, d1], F32, tag="outp")
                    nc.tensor.matmul(out=op_[:],
                                     lhsT=meanT[:, j * P:(j + 1) * P],
                                     rhs=wl0_t[:], start=True, stop=False)
                    nc.tensor.matmul(out=op_[:],
                                     lhsT=xT_t[:, t * P:(t + 1) * P],
                                     rhs=wr0_t[:], start=False, stop=False)
                    nc.tensor.matmul(out=op_[:], lhsT=ones_t[:],
                                     rhs=b0_t[:], start=False,
                                     stop=True, skip_group_check=True)
                    h_sb = wk.tile([P, d1], DT, tag="h_sb")
                    nc.scalar.activation(h_sb[:], op_[:], Act.Relu)
                    nc.sync.dma_start(out=h1own[t * P:(t + 1) * P, :],
                                      in_=h_sb[:])
                    for h in range(H1):
                        trp = psB.tile([P, P], DT, tag="trp")
                        nc.tensor.transpose(
                            out=trp[:], in_=h_sb[:, h * P:(h + 1) * P],
                            identity=ident_dt[:])
                        hT_sb = wk.tile([P, P], DT, tag="hT_sb")
                        nc.vector.tensor_copy(hT_sb[:], trp[:])
                        nc.sync.dma_start(
                            out=h1T[h * P:(h + 1) * P, t * P:(t + 1) * P],
                            in_=hT_sb[:])
                if gi == 15:
                    nc.gpsimd.collective_compute(
                        "AllGather", Alu.bypass,
                        replica_groups=[list(range(cfg.NC))],
                        ins=[h1own[0:cfg.SLA, :]], outs=[h1fullA.opt()])

            nc.gpsimd.collective_compute(
                "AllGather", Alu.bypass,
                replica_groups=[list(range(cfg.NC))],
                ins=[h1own[cfg.SLA:cfg.SHARD, :]], outs=[h1fullB.opt()])

            # ---------------- Layer 2 ----------------
            DEPTH = 3
            mAs = {}
            for gx in range(cfg.NPAIR + DEPTH):
                if gx < cfg.NPAIR:
                    mAs[gx] = gather1(gx, 0, h1fullA, d1, "mtA")
                gi = gx - DEPTH
                if gi < 0:
                    continue
                tl = plan.pairs[gi]
                mB = gather1(gi, 1, h1fullB, d1, "mtB")
                mA = mAs.pop(gi)
                s_t = smats(gi)
                ng = int(NAB[gi, 0] + NAB[gi, 1])
                aggs = []
                for j in range(len(tl)):
                    aggs.append(psA.tile([P, d1], F32, tag=f"agg{j}",
                                         name=f"agg2_{gi}_{j}"))
                for ci in range(ng):
                    for j in range(len(tl)):
                        nc.tensor.matmul(
                            out=aggs[j][:], lhsT=s_t[:, ci, j * P:(j + 1) * P],
                            rhs=chunk_of(gi, mA, mB, ci),
                            start=(ci == 0), stop=(ci == ng - 1))
                for j, t in enumerate(tl):
                    agg_sb = wk.tile([P, d1], DT, tag="agg_sb")
                    nc.scalar.activation(agg_sb[:], aggs[j][:], Act.Copy)
                    mts = []
                    for h in range(H1):
                        trp = psB.tile([P, P], DT, tag="trp")
                        nc.tensor.transpose(out=trp[:],
                                            in_=agg_sb[:, h * P:(h + 1) * P],
                                            identity=ident_dt[:])
                        mt_sb = wk.tile([P, P], DT, tag="mT2")
                        nc.vector.tensor_copy(mt_sb[:], trp[:])
                        mts.append(mt_sb)
                    h1T_t = wk.tile([P, H1, P], DT, tag="hTt")
                    for h in range(H1):
                        nc.sync.dma_start(
                            out=h1T_t[:, h, :],
                            in_=h1T[h * P:(h + 1) * P, t * P:(t + 1) * P])
                    op_ = psB.tile([P, d2], F32, tag="outp")
                    for h in range(H1):
                        nc.tensor.matmul(out=op_[:], lhsT=mts[h][:],
                                         rhs=wl1_t[:, h, :],
                                         start=(h == 0), stop=False)
                    for h in range(H1):
                        nc.tensor.matmul(out=op_[:], lhsT=h1T_t[:, h, :],
                                         rhs=wr1_t[:, h, :],
                                         start=False, stop=False)
                    nc.tensor.matmul(out=op_[:], lhsT=ones_t[:],
                                     rhs=b1_t[:], start=False, stop=True,
                                     skip_group_check=True)
                    h_sb = wk.tile([P, d2], DT, tag="h_sb")
                    nc.scalar.activation(h_sb[:], op_[:], Act.Relu)
                    hts = []
                    for h in range(H2):
                        trp = psB.tile([P, P], DT, tag="trp")
                        nc.tensor.transpose(out=trp[:],
                                            in_=h_sb[:, h * P:(h + 1) * P],
                                            identity=ident_dt[:])
                        hT_sb = wk.tile([P, P], DT, tag="hT_sb")
                        nc.vector.tensor_copy(hT_sb[:], trp[:])
                        nc.sync.dma_start(
                            out=h2T[h * P:(h + 1) * P, t * P:(t + 1) * P],
                            in_=hT_sb[:])
                        hts.append(hT_sb)
                    prj = psB.tile([P, d3], F32, tag="outp")
                    for h in range(H2):
                        nc.tensor.matmul(out=prj[:], lhsT=hts[h][:],
                                         rhs=wl2_t[:, h, :],
                                         start=(h == 0), stop=(h == H2 - 1))
                    prj_sb = wk.tile([P, d3], DT, tag="prj_sb")
                    nc.scalar.activation(prj_sb[:], prj[:], Act.Copy)
                    nc.sync.dma_start(out=h2p[t * P:(t + 1) * P, 0:d3],
                                      in_=prj_sb[:])
                if gi == 15:
                    nc.gpsimd.collective_compute(
                        "AllGather", Alu.bypass,
                        replica_groups=[list(range(cfg.NC))],
                        ins=[h2p[0:cfg.SLA, :]], outs=[h2pfullA.opt()])

            nc.gpsimd.collective_compute(
                "AllGather", Alu.bypass,
                replica_groups=[list(range(cfg.NC))],
                ins=[h2p[cfg.SLA:cfg.SHARD, :]], outs=[h2pfullB.opt()])

            # ---------------- Layer 3 ----------------
            mAs = {}
            for gx in range(cfg.NPAIR + DEPTH):
                if gx < cfg.NPAIR:
                    mAs[gx] = gather1(gx, 0, h2pfullA, cfg.EL3, "mtA")
                gi = gx - DEPTH
                if gi < 0:
                    continue
                tl = plan.pairs[gi]
                mB = gather1(gi, 1, h2pfullB, cfg.EL3, "mtB")
                mA = mAs.pop(gi)
                s_t = smats(gi)
                ng = int(NAB[gi, 0] + NAB[gi, 1])
                ops = []
                for j in range(len(tl)):
                    ops.append(psA.tile([P, d3], F32, tag=f"agg{j}",
                                        name=f"agg3_{gi}_{j}"))
                for ci in range(ng):
                    for j in range(len(tl)):
                        nc.tensor.matmul(
                            out=ops[j][:], lhsT=s_t[:, ci, j * P:(j + 1) * P],
                            rhs=(mA[:, ci, 0:d3] if ci < int(NAB[gi, 0])
                                 else mB[:, ci - int(NAB[gi, 0]), 0:d3]),
                            start=(ci == 0), stop=False)
                for j, t in enumerate(tl):
                    op_ = ops[j]
                    h2T_t = wk.tile([P, H2, P], DT, tag="hTt")
                    for h in range(H2):
                        nc.sync.dma_start(
                            out=h2T_t[:, h, :],
                            in_=h2T[h * P:(h + 1) * P, t * P:(t + 1) * P])
                    for h in range(H2):
                        nc.tensor.matmul(out=op_[:], lhsT=h2T_t[:, h, :],
                                         rhs=wr2_t[:, h, :],
                                         start=False, stop=False,
                                         skip_group_check=True)
                    nc.tensor.matmul(out=op_[:], lhsT=ones_t[:],
                                     rhs=b2_t[:], start=False, stop=True,
                                     skip_group_check=True)
                    # log_softmax over the d3 free dim
                    mx = wk.tile([P, 1], F32, tag="mx")
                    nc.vector.reduce_max(mx[:], op_[:],
                                         axis=mybir.AxisListType.X)
                    mxn = wk.tile([P, 1], F32, tag="mxn")
                    nc.vector.tensor_scalar(out=mxn[:], in0=mx[:],
                                            scalar1=-1.0, scalar2=None,
                                            op0=Alu.mult)
                    e_sb = wk.tile([P, d3], F32, tag="e_sb")
                    se = wk.tile([P, 1], F32, tag="se")
                    nc.scalar.activation(e_sb[:], op_[:], Act.Exp,
                                         bias=mxn[:, 0:1], accum_out=se[:])
                    ls = wk.tile([P, 1], F32, tag="ls")
                    nc.scalar.activation(ls[:], se[:], Act.Ln)
                    ofs = wk.tile([P, 1], F32, tag="ofs")
                    nc.vector.tensor_tensor(out=ofs[:], in0=mxn[:], in1=ls[:],
                                            op=Alu.subtract)
                    out_sb = wk.tile([P, d3], F32, tag="out_sb")
                    nc.vector.tensor_scalar(out=out_sb[:], in0=op_[:],
                                            scalar1=ofs[:, 0:1], scalar2=None,
                                            op0=Alu.add)
                    nc.sync.dma_start(out=out_t[t * P:(t + 1) * P, :],
                                      in_=out_sb[:])

    nc.compile()
    return nc


_NC_CACHE = {}


def get_nc(cfg, plan):
    key = (cfg.key(), plan.NAB.tobytes())
    if key not in _NC_CACHE:
        _NC_CACHE[key] = build_nc(cfg, plan)
    return _NC_CACHE[key]


def run(cfg, inputs, trace=False, tmpdir=None):
    x = np.asarray(inputs["x"], np.float32)
    plan, idx_full, s_full, assign = host_prep(
        cfg, x, np.asarray(inputs["edge_index"]),
        np.asarray(inputs["edge_attr"], np.float32))
    d0, d1, d2, d3 = cfg.D
    H1, H2 = d1 // P, d2 // P
    npDT = np_bf16 if cfg.bf16 else np.float32

    xpad = np.zeros((cfg.NPAD, d0), np.float32)
    xpad[:cfg.N] = x
    xpad = xpad.astype(npDT)
    Wl1 = np.asarray(inputs["Wl1"], np.float32)
    Wr1 = np.asarray(inputs["Wr1"], np.float32)
    Wl2 = np.asarray(inputs["Wl2"], np.float32)
    Wr2 = np.asarray(inputs["Wr2"], np.float32)
    # per-core slot-major x: rows of core k = tiles assign[k] in slot order
    row_of = (assign[:, :, None] * P
              + np.arange(P)[None, None, :]).reshape(cfg.NC, cfg.SHARD)
    xk = xpad[row_of]                       # [NC, SHARD, d0]
    xA = xk[:, :cfg.SLA].reshape(-1, d0)
    xB = xk[:, cfg.SLA:].reshape(-1, d0)
    shared = {
        "xfullA": np.ascontiguousarray(xA),
        "xfullB": np.ascontiguousarray(xB),
        "wl0": np.asarray(inputs["Wl0"], np.float32).astype(npDT),
        "wr0": np.asarray(inputs["Wr0"], np.float32).astype(npDT),
        "wl1": Wl1.reshape(H1, P, d2).transpose(1, 0, 2).astype(npDT),
        "wr1": Wr1.reshape(H1, P, d2).transpose(1, 0, 2).astype(npDT),
        "wl2": Wl2.reshape(H2, P, d3).transpose(1, 0, 2).astype(npDT),
        "wr2": Wr2.reshape(H2, P, d3).transpose(1, 0, 2).astype(npDT),
        "b0": (np.asarray(inputs["bl0"]) + np.asarray(inputs["br0"]))
        .astype(np.float32)[None, :].astype(npDT),
        "b1": (np.asarray(inputs["bl1"]) + np.asarray(inputs["br1"]))
        .astype(np.float32)[None, :].astype(npDT),
        "b2": (np.asarray(inputs["bl2"]) + np.asarray(inputs["br2"]))
        .astype(np.float32)[None, :].astype(npDT),
    }
    in_maps = []
    for k in range(cfg.NC):
        in_maps.append({
            **shared,
            "xT": np.ascontiguousarray(xk[k].T),
            "idx": idx_full[k],
            "smat": s_full[k],
        })
    nc = get_nc(cfg, plan)
    res = run_bass_kernel_spmd(nc, in_maps, core_ids=list(range(cfg.NC)),
                               trace=trace, tmpdir=tmpdir)
    out = np.empty((cfg.NPAD, d3), np.float32)
    for k in range(cfg.NC):
        out[row_of[k]] = np.asarray(res.results[k]["out"], np.float32)
    return np.ascontiguousarray(out[:cfg.N]), res


def kernel(**inputs):
    cfg = Cfg()
    out, _ = run(cfg, inputs)
    return out
